# revision 15
# baseline (speedup 1.0000x reference)
"""Multi-LoRA routed adapter kernel for Trainium2 (8 NeuronCores).

Problem: out[b] = (x[b] @ B[aid[b]].T) @ A[aid[b]].T * (alpha/rank)
  x: [8, 1024, 2048] f32, A: [8, 2048, 16] f32, B: [8, 16, 2048] f32,
  adapter_ids: [8] i32, alpha/rank = 16/16 = 1.0.

Strategy: data-parallel over batch — sample b runs on core b. The
adapter gather (routing) is resolved host-side: each core receives only
its sample's selected A/B, pre-transposed so all device DMAs are
contiguous and the contraction dims land on SBUF partitions.

Per-core device kernel (two chained matmuls, fp32r = full-rate fp32
with 11 mantissa bits):
  matmul1: BxT[r, n]  = sum_i  B^T[i, r]^T  @ xT[i, n]   (K=i tiled by 128)
  matmul2: y[n, o]    = sum_r  BxT[r, n]^T  @ AT[r, o]   (K=r=16)

Constraint shaping: the fused fp32r matmul carries at most ONE HW
sync-wait, so the program is arranged so every matmul depends on a
single semaphore "processor":
  - B^T is concatenated into each x chunk host-side -> matmul1's two
    operands come from ONE DMA.
  - AT reaches SBUF via a DVE cast-copy, so matmul2's deps (AT, BxT,
    psum-reuse) all sit on the DVE clock.
x is pre-rounded to fp32r host-side (RNE at 11 mantissa bits), so the
DMAs are cast-free HWDGE transfers.
"""

import os

import numpy as np

import concourse.bass as bass
import concourse.mybir as mybir
import concourse.tile as tile
from concourse import bacc
from concourse.bass_utils import run_bass_kernel_spmd

# Problem constants (hardcoded per spec).
N_CORES = 8
BATCH = 8
N_TOK = 1024
D_IN = 2048
D_OUT = 2048
RANK = 16
SCALING = 16.0 / 16.0  # alpha / rank

P = 128
K_TILES = D_IN // P  # 16
N_CHUNK = 512
N_CHUNKS = N_TOK // N_CHUNK  # keep == psbx bufs (avoids a 2nd matmul wait)
BLK = N_CHUNK + RANK  # x chunk + B^T columns, interleaved host-side
O_CHUNK = 512

F32 = mybir.dt.float32
F32R = mybir.dt.float32r

_last_results = None  # stashed BassKernelResults for test harness introspection


def _round_fp32r(a: np.ndarray) -> np.ndarray:
    """Round fp32 to fp32r (sign/8-bit exp/11-bit mantissa), RNE.

    Matches walrus's fp32_to_fp32r: values keep only the top 11 mantissa
    bits; carries may propagate into the exponent (correct RNE).
    """
    u = np.ascontiguousarray(a, dtype=np.float32).view(np.uint32)
    lsb = (u >> np.uint32(12)) & np.uint32(1)
    u = u + np.uint32(0x7FF) + lsb
    u &= np.uint32(0xFFFFF000)
    return u.view(np.float32)


def _build_nc() -> bass.Bass:
    # Bacc (not plain Bass): its compile() pipeline legalizes multi-semaphore
    # waits (move_matmul_waits_to_ldweights, replace_nops_with_events), which
    # walrus requires — raw Tile output can exceed per-instruction wait slots.
    nc = bacc.Bacc(None)
    xaug = nc.dram_tensor("xaug", [D_IN, N_CHUNKS * BLK], F32R, kind="ExternalInput")
    AT = nc.dram_tensor("AT", [RANK, D_OUT], F32, kind="ExternalInput")
    y = nc.dram_tensor("y", [N_TOK, D_OUT], F32, kind="ExternalOutput")

    with tile.TileContext(nc) as tc:
        with (
            tc.tile_pool(name="const", bufs=1) as cpool,
            tc.tile_pool(name="xin", bufs=1) as xpool,
            tc.tile_pool(name="bx", bufs=2) as bxpool,
            tc.tile_pool(name="outb", bufs=2) as opool,
            tc.tile_pool(name="psbx", bufs=N_CHUNKS, space="PSUM") as psbx,
            tc.tile_pool(name="pso", bufs=6, space="PSUM") as pso,
        ):
            at_f32 = cpool.tile([RANK, D_OUT], F32)
            nc.sync.dma_start(at_f32[:], AT[:, :])
            at_sb = cpool.tile([RANK, D_OUT], F32R)
            nc.vector.tensor_copy(at_sb[:], at_f32[:])  # DVE rounds to fp32r

            xaug_r = xaug.rearrange("(kt p) m -> p kt m", p=P)  # [128, 16, *]
            x_sb = xpool.tile([P, K_TILES, N_CHUNKS * BLK], F32R)
            nc.sync.dma_start(x_sb[:], xaug_r[:, :, :])

            for ch in range(N_CHUNKS):
                blk0 = ch * BLK
                ps_bx = psbx.tile([RANK, N_CHUNK], F32)
                for kt in range(K_TILES):
                    nc.tensor.matmul(
                        ps_bx[:],
                        x_sb[:, kt, blk0 + N_CHUNK : blk0 + BLK],  # B^T [128, 16]
                        x_sb[:, kt, blk0 : blk0 + N_CHUNK],  # x^T [128, 512]
                        start=(kt == 0),
                        stop=(kt == K_TILES - 1),
                    )
                bx_sb = bxpool.tile([RANK, N_CHUNK], F32R)
                nc.vector.tensor_copy(bx_sb[:], ps_bx[:])

                o_sb = opool.tile([P, N_CHUNK // P, D_OUT], F32)
                for ns in range(N_CHUNK // P):
                    for oc in range(D_OUT // O_CHUNK):
                        ps_o = pso.tile([P, O_CHUNK], F32)
                        nc.tensor.matmul(
                            ps_o[:],
                            bx_sb[:, ns * P : (ns + 1) * P],
                            at_sb[:, oc * O_CHUNK : (oc + 1) * O_CHUNK],
                            start=True,
                            stop=True,
                        )
                        nc.vector.tensor_copy(
                            o_sb[:, ns, oc * O_CHUNK : (oc + 1) * O_CHUNK], ps_o[:]
                        )
                # One out-DMA per chunk: y rows [ch*512, (ch+1)*512).
                y_chunk = y[ch * N_CHUNK : (ch + 1) * N_CHUNK, :].rearrange(
                    "(ns p) o -> p ns o", p=P
                )
                nc.sync.dma_start(y_chunk, o_sb[:])
    nc.compile()
    return nc


def kernel(x, A, B, adapter_ids):
    global _last_results
    x = np.asarray(x, dtype=np.float32)
    A = np.asarray(A, dtype=np.float32)
    B = np.asarray(B, dtype=np.float32)
    adapter_ids = np.asarray(adapter_ids)

    assert x.shape == (BATCH, N_TOK, D_IN)

    in_maps = []
    for b in range(BATCH):
        aid = int(adapter_ids[b])
        # Fold the LoRA scaling into A (scaling is 1.0 here, exact).
        At = np.ascontiguousarray(A[aid].T * np.float32(SCALING))  # [16, 2048]
        Bt = B[aid].T  # [2048, 16]
        xbT = x[b].T  # [2048, 1024]
        # Per chunk: [512 x^T columns | 16 B^T columns], so matmul1's two
        # operands arrive in one DMA.
        blocks = []
        for ch in range(N_CHUNKS):
            blocks.append(xbT[:, ch * N_CHUNK : (ch + 1) * N_CHUNK])
            blocks.append(Bt)
        xaug = _round_fp32r(np.concatenate(blocks, axis=1))  # [2048, N_CHUNKS*528]
        in_maps.append({"xaug": xaug, "AT": At})

    nc = _build_nc()
    trace = bool(int(os.environ.get("KERNEL_BASS_TRACE", "0")))
    res = run_bass_kernel_spmd(
        nc, in_maps, core_ids=list(range(N_CORES)), trace=trace
    )
    _last_results = res

    out = np.empty((BATCH, N_TOK, D_OUT), dtype=np.float32)
    for b in range(BATCH):
        out[b] = res.results[b]["y"]
    return out


# revision 17
# speedup vs baseline: 1.3318x; 1.3318x over previous
"""Multi-LoRA routed adapter kernel for Trainium2 (8 NeuronCores).

Problem: out[b] = (x[b] @ B[aid[b]].T) @ A[aid[b]].T * (alpha/rank)
  x: [8, 1024, 2048] f32, A: [8, 2048, 16] f32, B: [8, 16, 2048] f32,
  adapter_ids: [8] i32, alpha/rank = 16/16 = 1.0.

Strategy: data-parallel over batch — sample b runs on core b. The
adapter gather (routing) is resolved host-side: each core receives only
its sample's selected A/B, pre-transposed so all device DMAs are
contiguous and the contraction dims land on SBUF partitions.

Per-core device kernel (two chained matmuls, fp32r = full-rate fp32
with 11 mantissa bits):
  matmul1: BxT[r, n]  = sum_i  B^T[i, r]^T  @ xT[i, n]   (K=i tiled by 128)
  matmul2: y[n, o]    = sum_r  BxT[r, n]^T  @ AT[r, o]   (K=r=16)

Constraint shaping: the fused fp32r matmul carries at most ONE HW
sync-wait, so the program is arranged so every matmul depends on a
single semaphore "processor":
  - B^T is concatenated into each x chunk host-side -> matmul1's two
    operands come from ONE DMA.
  - AT reaches SBUF via a DVE cast-copy, so matmul2's deps (AT, BxT,
    psum-reuse) all sit on the DVE clock.
x is pre-rounded to fp32r host-side (RNE at 11 mantissa bits), so the
DMAs are cast-free HWDGE transfers.
"""

import os

import numpy as np

import concourse.bass as bass
import concourse.mybir as mybir
import concourse.tile as tile
from concourse import bacc
from concourse.bass_utils import run_bass_kernel_spmd

# Problem constants (hardcoded per spec).
N_CORES = 8
BATCH = 8
N_TOK = 1024
D_IN = 2048
D_OUT = 2048
RANK = 16
SCALING = 16.0 / 16.0  # alpha / rank

P = 128
K_TILES = D_IN // P  # 16
N_CHUNK = 256  # >=256 keeps fp32r matmul1 at full rate (1 cyc/row)
N_CHUNKS = N_TOK // N_CHUNK
BLK = N_CHUNK + RANK  # x chunk + B^T columns, interleaved host-side
O_CHUNK = 512

F32 = mybir.dt.float32
F32R = mybir.dt.float32r

_last_results = None  # stashed BassKernelResults for test harness introspection


def _round_fp32r(a: np.ndarray) -> np.ndarray:
    """Round fp32 to fp32r (sign/8-bit exp/11-bit mantissa), RNE.

    Matches walrus's fp32_to_fp32r: values keep only the top 11 mantissa
    bits; carries may propagate into the exponent (correct RNE).
    """
    u = np.ascontiguousarray(a, dtype=np.float32).view(np.uint32)
    lsb = (u >> np.uint32(12)) & np.uint32(1)
    u = u + np.uint32(0x7FF) + lsb
    u &= np.uint32(0xFFFFF000)
    return u.view(np.float32)


def _build_nc() -> bass.Bass:
    # Bacc (not plain Bass): its compile() pipeline legalizes multi-semaphore
    # waits (move_matmul_waits_to_ldweights, replace_nops_with_events), which
    # walrus requires — raw Tile output can exceed per-instruction wait slots.
    nc = bacc.Bacc(None)
    xaug = nc.dram_tensor("xaug", [D_IN, N_CHUNKS * BLK], F32R, kind="ExternalInput")
    AT = nc.dram_tensor("AT", [RANK, D_OUT], F32, kind="ExternalInput")
    y = nc.dram_tensor("y", [N_TOK, D_OUT], F32, kind="ExternalOutput")

    with tile.TileContext(nc) as tc:
        with (
            tc.tile_pool(name="const", bufs=1) as cpool,
            tc.tile_pool(name="xin", bufs=3) as xpool,
            tc.tile_pool(name="bx", bufs=2) as bxpool,
            tc.tile_pool(name="outb", bufs=3) as opool,
            tc.tile_pool(name="psbx", bufs=2, space="PSUM") as psbx,
            tc.tile_pool(name="pso", bufs=6, space="PSUM") as pso,
        ):
            at_f32 = cpool.tile([RANK, D_OUT], F32)
            nc.sync.dma_start(at_f32[:], AT[:, :])
            at_sb = cpool.tile([RANK, D_OUT], F32R)
            nc.vector.tensor_copy(at_sb[:], at_f32[:])  # DVE rounds to fp32r

            xaug_r = xaug.rearrange("(kt p) m -> p kt m", p=P)  # [128, 16, *]

            # Loads ride the SP HWDGE ring; stores ride the ACT ring. A
            # store waiting on compute would otherwise block later loads
            # (HWDGE rings are FIFO at the sequencer's semaphore wait).
            for ch in range(N_CHUNKS):
                x_sb = xpool.tile([P, K_TILES, BLK], F32R)
                nc.sync.dma_start(x_sb[:], xaug_r[:, :, ch * BLK : (ch + 1) * BLK])

                ps_bx = psbx.tile([RANK, N_CHUNK], F32)
                for kt in range(K_TILES):
                    nc.tensor.matmul(
                        ps_bx[:],
                        x_sb[:, kt, N_CHUNK:BLK],  # B^T [128, 16]
                        x_sb[:, kt, 0:N_CHUNK],  # x^T [128, 256]
                        start=(kt == 0),
                        stop=(kt == K_TILES - 1),
                    )
                bx_sb = bxpool.tile([RANK, N_CHUNK], F32R)
                nc.vector.tensor_copy(bx_sb[:], ps_bx[:])

                o_sb = opool.tile([P, N_CHUNK // P, D_OUT], F32)
                for ns in range(N_CHUNK // P):
                    for oc in range(D_OUT // O_CHUNK):
                        ps_o = pso.tile([P, O_CHUNK], F32)
                        nc.tensor.matmul(
                            ps_o[:],
                            bx_sb[:, ns * P : (ns + 1) * P],
                            at_sb[:, oc * O_CHUNK : (oc + 1) * O_CHUNK],
                            start=True,
                            stop=True,
                        )
                        nc.vector.tensor_copy(
                            o_sb[:, ns, oc * O_CHUNK : (oc + 1) * O_CHUNK], ps_o[:]
                        )
                # One out-DMA per chunk on the scalar (ACT) HWDGE ring.
                y_chunk = y[ch * N_CHUNK : (ch + 1) * N_CHUNK, :].rearrange(
                    "(ns p) o -> p ns o", p=P
                )
                nc.scalar.dma_start(y_chunk, o_sb[:])
    nc.compile()
    return nc


def kernel(x, A, B, adapter_ids):
    global _last_results
    x = np.asarray(x, dtype=np.float32)
    A = np.asarray(A, dtype=np.float32)
    B = np.asarray(B, dtype=np.float32)
    adapter_ids = np.asarray(adapter_ids)

    assert x.shape == (BATCH, N_TOK, D_IN)

    in_maps = []
    for b in range(BATCH):
        aid = int(adapter_ids[b])
        # Fold the LoRA scaling into A (scaling is 1.0 here, exact).
        At = np.ascontiguousarray(A[aid].T * np.float32(SCALING))  # [16, 2048]
        Bt = B[aid].T  # [2048, 16]
        xbT = x[b].T  # [2048, 1024]
        # Per chunk: [512 x^T columns | 16 B^T columns], so matmul1's two
        # operands arrive in one DMA.
        blocks = []
        for ch in range(N_CHUNKS):
            blocks.append(xbT[:, ch * N_CHUNK : (ch + 1) * N_CHUNK])
            blocks.append(Bt)
        xaug = _round_fp32r(np.concatenate(blocks, axis=1))  # [2048, N_CHUNKS*528]
        in_maps.append({"xaug": xaug, "AT": At})

    nc = _build_nc()
    trace = bool(int(os.environ.get("KERNEL_BASS_TRACE", "0")))
    res = run_bass_kernel_spmd(
        nc, in_maps, core_ids=list(range(N_CORES)), trace=trace
    )
    _last_results = res

    out = np.empty((BATCH, N_TOK, D_OUT), dtype=np.float32)
    for b in range(BATCH):
        out[b] = res.results[b]["y"]
    return out
